# revision 26
# baseline (speedup 1.0000x reference)
"""Causal self-attention Trainium2 kernel.

B=4, T=2048, C=1024, H=16 heads, D=64. 8 NeuronCores, tensor-parallel over
heads: core c owns heads {2c, 2c+1}. Host pre-transposes x to xT [C, B*T],
column-shards W_attn / row-shards W_proj, sums the 8 partial outputs.

Device kernel (per core, SPMD), software-pipelined by batch so the PE-heavy
qkv phase of batch b+1 overlaps the ACT-heavy attention phase of batch b:
  qkv:  qkvT[384, T] = W_core.T @ xT  (fp32r matmuls, K=C in 8 chunks)
        rows: [qA qB | kA kB | vA vB], 64 each. Bias added on eviction (DVE).
        v rows are PE-transposed to token-major V with an appended ones
        column (gives softmax denominators for free in the y matmul).
  attn: S^T layout: S^T[keys,queries] = k @ q^T via matmul(lhsT=kT_chunk,
        rhs=qT_block); the two heads' K=64 matmuls are issued adjacently so
        the PE can run them concurrently in different row groups. exp on ACT
        over pairs of key chunks; causal mask via gpsimd affine_select;
        y^T[d, queries] accumulated via matmul(lhsT=[V|1], rhs=P^T);
        normalization via DVE reciprocal + gpsimd partition_broadcast.
  proj: partial out[tokens, C] = y^T.T @ W_proj_rows, DMA'd out.
"""

import sys

sys.path.insert(0, "/opt/trn_rl_repo")

from contextlib import ExitStack

import numpy as np

import concourse.bass as bass
import concourse.mybir as mybir
import concourse.tile as tile
from concourse import bacc
from concourse.bass_utils import run_bass_kernel_spmd
from concourse.masks import make_identity

F32 = mybir.dt.float32
F32R = mybir.dt.float32r
BF16 = mybir.dt.bfloat16
AF = mybir.ActivationFunctionType

B, T, C, H, D = 4, 2048, 1024, 16, 64
NCORES = 8
HPC = H // NCORES  # heads per core = 2
TOK = B * T  # 8192
QKVC = HPC * D  # per-core channels per q/k/v = 128
TB = 256  # token block for the qkv phase
NBB = T // TB  # qkv token blocks per batch = 8
QB = 512  # query block for attention
NKC = T // 128  # key chunks per batch = 16
SCALE = 1.0 / 8.0  # 1/sqrt(D)


def build_program():
    nc = bacc.Bacc(
        "TRN2",
        target_bir_lowering=False,
        debug=False,
        num_devices=NCORES,
    )
    xt_d = nc.dram_tensor("xt", [C, TOK], BF16, kind="ExternalInput").ap()
    wqkv_d = nc.dram_tensor("wqkv", [C, 3 * QKVC], BF16, kind="ExternalInput").ap()
    bqkv_d = nc.dram_tensor("bqkv", [3 * QKVC], F32, kind="ExternalInput").ap()
    wproj_d = nc.dram_tensor("wproj", [QKVC, C], F32R, kind="ExternalInput").ap()
    outp_d = nc.dram_tensor("outp", [TOK, C], F32, kind="ExternalOutput").ap()

    with tile.TileContext(nc) as tc:
        with ExitStack() as ctx, nc.allow_low_precision(reason="fp32r matmul inputs"):
            _body(ctx, tc, xt_d, wqkv_d, bqkv_d, wproj_d, outp_d)
    nc.compile()
    return nc


class _Kern:
    def __init__(self, ctx, tc, xt_d, wqkv_d, bqkv_d, wproj_d, outp_d):
        nc = tc.nc
        self.nc = nc
        self.tc = tc
        self.outp_d = outp_d

        self.const = ctx.enter_context(tc.tile_pool(name="const", bufs=1))
        self.persist = ctx.enter_context(tc.tile_pool(name="persist", bufs=1))
        self.xt_pool = ctx.enter_context(tc.tile_pool(name="xt", bufs=2))
        self.vtmp_pool = ctx.enter_context(tc.tile_pool(name="vtmp", bufs=2))
        self.pt_pool = ctx.enter_context(tc.tile_pool(name="pt", bufs=8))
        self.yt_pool = ctx.enter_context(tc.tile_pool(name="yt", bufs=2))
        self.out_pool = ctx.enter_context(tc.tile_pool(name="osb", bufs=3))
        self.small_pool = ctx.enter_context(tc.tile_pool(name="small", bufs=4))
        self.bc_pool = ctx.enter_context(tc.tile_pool(name="bc", bufs=4))

        self.ps_s = ctx.enter_context(tc.tile_pool(name="ps_s", bufs=2, space="PSUM"))
        self.ps_y = ctx.enter_context(tc.tile_pool(name="ps_y", bufs=2, space="PSUM"))
        self.ps_mm = ctx.enter_context(tc.tile_pool(name="ps_mm", bufs=2, space="PSUM"))

        # --- constants ---
        c = self.const
        # weight loads ride the ACT HWDGE ring (nc.scalar) so they don't
        # serialize with the xt streaming loads on the SP ring; the first
        # K-chunk is split out so the PE can start quickly
        self.wqkv_s = c.tile([128, 8, 3 * QKVC], BF16, tag="wqkv", name="wqkv_s")
        wqkv_r = wqkv_d.rearrange("(kc p) m -> p kc m", p=128)
        for kc in range(8):
            nc.scalar.dma_start(self.wqkv_s[:, kc : kc + 1, :], wqkv_r[:, kc : kc + 1, :])
        self.bqkv_s = c.tile([128, 3], F32, tag="bqkv", name="bqkv_s")
        nc.scalar.dma_start(self.bqkv_s[:], bqkv_d.rearrange("(m p) -> p m", p=128))
        self.wproj_s = c.tile([128, C], F32R, tag="wproj", name="wproj_s")
        nc.scalar.dma_start(self.wproj_s[:], wproj_d[:])
        self.ident = c.tile([128, 128], F32, tag="ident", name="ident")
        make_identity(nc, self.ident[:])

        # persistent activations
        self.qT = self.persist.tile([128, TOK], F32R, tag="qT", name="qT")
        self.kT = self.persist.tile([128, TOK], F32R, tag="kT", name="kT")
        # token-major V (cols 0:D) + 64 replicated ones columns (cols D:2D):
        # the y^T matmul then yields the softmax denominator replicated on
        # psum partitions D..2D, row-aligned with y for the normalize mul
        self.vones = self.persist.tile(
            [128, B, HPC, NKC, 2 * D], BF16, tag="vones", name="vones"
        )
        nc.gpsimd.memset(self.vones[:, :, :, :, D : 2 * D], 1.0)
        self.xt_r = xt_d.rearrange("(kc p) t -> p kc t", p=128)

    def qkv_block(self, b, nb):
        """QKV + V-transpose for token block nb (TB tokens) of batch b."""
        nc = self.nc
        n = b * NBB + nb
        xt_t = self.xt_pool.tile([128, 8, TB], BF16, tag="xt", name=f"xt{n}")
        if n == 0:
            # cold start: split the first load so the PE can start sooner
            for kc in range(8):
                nc.sync.dma_start(
                    xt_t[:, kc, :], self.xt_r[:, kc, n * TB : (n + 1) * TB]
                )
        else:
            nc.sync.dma_start(xt_t[:], self.xt_r[:, :, n * TB : (n + 1) * TB])
        for m in range(3):  # q, k, v row chunks
            ps = self.ps_mm.tile([128, TB], F32, tag="mm", name=f"qkvp{n}_{m}")
            for kc in range(8):
                nc.tensor.matmul(
                    ps[:],
                    self.wqkv_s[:, kc, m * 128 : (m + 1) * 128],
                    xt_t[:, kc, :],
                    start=(kc == 0),
                    stop=(kc == 7),
                )
            if m < 2:
                dst = (self.qT if m == 0 else self.kT)[:, n * TB : (n + 1) * TB]
                nc.vector.tensor_scalar_add(dst, ps[:], self.bqkv_s[:, m : m + 1])
            else:
                vt = self.vtmp_pool.tile([128, TB], F32, tag="vt", name=f"vt{n}")
                nc.vector.tensor_scalar_add(vt[:], ps[:], self.bqkv_s[:, 2:3])
                j0 = (TB // 128) * nb
                for jj in range(TB // 128):
                    pst = self.ps_mm.tile([128, 128], F32, tag="mm", name=f"tr{n}_{jj}")
                    nc.tensor.transpose(
                        pst[:], vt[:, jj * 128 : (jj + 1) * 128], self.ident[:]
                    )
                    nc.vector.tensor_copy(
                        self.vones[:, b, :, j0 + jj, 0:D],
                        pst[:].rearrange("p (h d) -> p h d", h=HPC),
                    )

    def attn_block(self, b, qb):
        """Attention + proj for query block qb (QB queries) of batch b."""
        nc = self.nc
        q0 = b * T + qb * QB
        nj = (qb + 1) * (QB // 128)  # key chunks attended by this block
        psy = [
            self.ps_y.tile([2 * D, QB], F32, tag="psy", name=f"psy{b}_{qb}_{h}")
            for h in range(HPC)
        ]
        for j in range(nj):  # key chunks of 128
            k0 = b * T + j * 128
            # diagonal trimming: for a diagonal chunk at offset d, queries
            # f < 128*d attend to no key in this chunk, so restrict all work
            # to the query range [f0, QB)
            d = j - (nj - 4)
            f0 = 128 * d if d > 0 else 0
            fw = QB - f0
            # one 2-bank psum tile holds both heads' S^T for this chunk;
            # the two K=64 matmuls use partitions 0-63 / 64-127 -> different
            # PE row groups, issued adjacently so they can run concurrently
            ps2 = self.ps_s.tile([128, HPC, QB], F32, tag="s2", name=f"s{b}_{qb}_{j}")
            for h in range(HPC):
                nc.tensor.matmul(
                    ps2[:, h, f0:QB],
                    self.kT[h * D : (h + 1) * D, k0 : k0 + 128],
                    self.qT[h * D : (h + 1) * D, q0 + f0 : q0 + QB],
                    start=True,
                    stop=True,
                )
            pt = self.pt_pool.tile([128, HPC, QB], BF16, tag="pt", name=f"pt{b}_{qb}_{j}")
            nc.scalar.activation(pt[:, :, f0:QB], ps2[:, :, f0:QB], AF.Exp, scale=SCALE)
            if d >= 0:
                # mask only the 128-column window straddling the diagonal,
                # per head so the first yT matmul isn't gated on both
                cols = min(QB, 128 * (d + 1))
                for h in range(HPC):
                    nc.gpsimd.affine_select(
                        out=pt[:, h, f0:cols],
                        in_=pt[:, h, f0:cols],
                        base=QB * qb - 128 * j + f0,
                        channel_multiplier=-1,
                        pattern=[[1, cols - f0]],
                        compare_op=mybir.AluOpType.is_ge,
                        fill=0.0,
                    )
            for h in range(HPC):
                nc.tensor.matmul(
                    psy[h][:, f0:QB],
                    self.vones[:, b, h, j, :],
                    pt[:, h, f0:QB],
                    start=(j == 0),
                    stop=(j == nj - 1),
                )
        # normalize into yt (d-major, both heads stacked)
        yt = self.yt_pool.tile([128, QB], F32R, tag="yt", name=f"yt{b}_{qb}")
        for h in range(HPC):
            rec = self.small_pool.tile([D, QB], F32, tag="rec", name=f"rec{b}_{qb}_{h}")
            nc.vector.reciprocal(rec[:], psy[h][D : 2 * D, :])
            nc.vector.tensor_mul(yt[h * D : (h + 1) * D, :], psy[h][0:D, :], rec[:])
        return yt

    def proj_block(self, b, qb, yt):
        """Projection + output DMA for query block qb of batch b."""
        nc = self.nc
        q0 = b * T + qb * QB
        for tt in range(QB // 128):
            osb = self.out_pool.tile([128, C], F32, tag="osb", name=f"o{b}_{qb}_{tt}")
            for ncol in range(C // 512):
                po = self.ps_mm.tile([128, 512], F32, tag="mm", name=f"po{b}_{qb}_{tt}_{ncol}")
                nc.tensor.matmul(
                    po[:],
                    yt[:, tt * 128 : (tt + 1) * 128],
                    self.wproj_s[:, ncol * 512 : (ncol + 1) * 512],
                    start=True,
                    stop=True,
                )
                nc.vector.tensor_copy(osb[:, ncol * 512 : (ncol + 1) * 512], po[:])
            r0 = q0 + tt * 128
            nc.sync.dma_start(self.outp_d[r0 : r0 + 128, :], osb[:])


def _body(ctx, tc, xt_d, wqkv_d, bqkv_d, wproj_d, outp_d):
    k = _Kern(ctx, tc, xt_d, wqkv_d, bqkv_d, wproj_d, outp_d)
    # Software pipeline one batch deep: attention(b) interleaves with the
    # independent qkv(b+1) blocks so the PE always has ready matmuls while
    # ACT paces the softmax. proj is deferred one attention block so the PE
    # has ready work while the softmax-normalize chain completes.
    # qkv(0) is the prologue; batches 1..B-1 form a queue drained 2 blocks
    # per attention slot for the first half, then 1, so every attention
    # stretch (including the last batch's) has PE-dense qkv filler.
    pending = None
    for nb in range(NBB):
        k.qkv_block(0, nb)
    queue = [(b, nb) for b in range(1, B) for nb in range(NBB)]
    qi = 0
    nslots = B * (T // QB)
    for s in range(nslots):
        b, qb = s // (T // QB), s % (T // QB)
        want = 2 if s < nslots // 2 else 1
        # never emit attn before its qkv blocks: need batch b block 2qb+1
        need = 0 if b == 0 else (b - 1) * NBB + 2 * qb + 2
        while qi < len(queue) and (qi < need or want > 0):
            k.qkv_block(*queue[qi])
            qi += 1
            want -= 1
        yt = k.attn_block(b, qb)
        if pending is not None:
            k.proj_block(*pending)
        pending = (b, qb, yt)
    while qi < len(queue):
        k.qkv_block(*queue[qi])
        qi += 1
    k.proj_block(*pending)


_CACHED_NC = None


def _get_nc():
    global _CACHED_NC
    if _CACHED_NC is None:
        _CACHED_NC = build_program()
    return _CACHED_NC


def make_in_maps(x, W_attn, b_attn, W_proj):
    x = np.ascontiguousarray(np.asarray(x, dtype=np.float32))
    W_attn = np.asarray(W_attn, dtype=np.float32)
    b_attn = np.asarray(b_attn, dtype=np.float32)
    W_proj = np.asarray(W_proj, dtype=np.float32)
    import ml_dtypes

    xt = np.ascontiguousarray(x.reshape(TOK, C).T.astype(ml_dtypes.bfloat16))
    in_maps = []
    for c in range(NCORES):
        s = c * QKVC
        wq = W_attn[:, s : s + QKVC]
        wk = W_attn[:, C + s : C + s + QKVC]
        wv = W_attn[:, 2 * C + s : 2 * C + s + QKVC]
        wqkv = np.ascontiguousarray(
            np.concatenate([wq, wk, wv], axis=1).astype(ml_dtypes.bfloat16)
        )
        bq = b_attn[s : s + QKVC]
        bk = b_attn[C + s : C + s + QKVC]
        bv = b_attn[2 * C + s : 2 * C + s + QKVC]
        bqkv = np.ascontiguousarray(np.concatenate([bq, bk, bv]))
        wproj = np.ascontiguousarray(W_proj[s : s + QKVC, :])
        in_maps.append({"xt": xt, "wqkv": wqkv, "bqkv": bqkv, "wproj": wproj})
    return in_maps


def run(x, W_attn, b_attn, W_proj, b_proj, trace=False, **kwargs):
    nc = _get_nc()
    in_maps = make_in_maps(x, W_attn, b_attn, W_proj)
    res = run_bass_kernel_spmd(
        nc, in_maps, core_ids=list(range(NCORES)), trace=trace, **kwargs
    )
    acc = res.results[0]["outp"].astype(np.float32, copy=True)
    for c in range(1, NCORES):
        acc += res.results[c]["outp"]
    acc += np.asarray(b_proj, dtype=np.float32)[None, :]
    out = acc.reshape(B, T, C)
    return out, res


def kernel(x, W_attn, b_attn, W_proj, b_proj):
    out, _ = run(x, W_attn, b_attn, W_proj, b_proj, trace=False)
    return out


# revision 32
# speedup vs baseline: 1.0173x; 1.0173x over previous
"""Causal self-attention Trainium2 kernel.

B=4, T=2048, C=1024, H=16 heads, D=64. 8 NeuronCores, tensor-parallel over
heads: core c owns heads {2c, 2c+1}. Host pre-transposes x to xT [C, B*T],
column-shards W_attn / row-shards W_proj, sums the 8 partial outputs.

Device kernel (per core, SPMD), software-pipelined one batch deep so the
PE-heavy qkv matmuls fill the gaps of the ACT-paced attention stream:
  qkv:  qkvT[384, T] = W_core.T @ xT  (bf16 matmuls, K=C in 8 chunks);
        rows: [qA qB | kA kB | vA vB], 64 each. Bias added on eviction (DVE).
        v rows are PE-transposed to token-major V with 64 appended ones
        columns, so the y^T matmul emits the softmax denominators
        replicated on psum partitions 64-127, row-aligned with y.
  attn: S^T layout: S^T[keys,queries] = k @ q^T via matmul(lhsT=kT_chunk,
        rhs=qT_block, fp32r); the two heads' K=64 matmuls are issued
        adjacently so the PE can run them concurrently in different row
        groups. exp on ACT over both heads at once ([128, 2, 512] psum);
        causal mask via per-head column-restricted gpsimd affine_select;
        diagonal chunks restrict all work to the live query range.
        y^T[d, queries] accumulated via matmul(lhsT=[V|1s], rhs=P^T, bf16);
        normalization is recip + row-aligned mul on DVE.
  proj: partial out[tokens, C] = y^T.T @ W_proj_rows (fp32r), DMA'd out;
        deferred one block so the PE has ready work during normalize.
"""

import sys

sys.path.insert(0, "/opt/trn_rl_repo")

from contextlib import ExitStack

import numpy as np

import concourse.bass as bass
import concourse.mybir as mybir
import concourse.tile as tile
from concourse import bacc
from concourse.bass_utils import run_bass_kernel_spmd
from concourse.masks import make_identity

F32 = mybir.dt.float32
F32R = mybir.dt.float32r
BF16 = mybir.dt.bfloat16
AF = mybir.ActivationFunctionType

B, T, C, H, D = 4, 2048, 1024, 16, 64
NCORES = 8
HPC = H // NCORES  # heads per core = 2
TOK = B * T  # 8192
QKVC = HPC * D  # per-core channels per q/k/v = 128
TB = 256  # token block for the qkv phase
NBB = T // TB  # qkv token blocks per batch = 8
QB = 512  # query block for attention
NKC = T // 128  # key chunks per batch = 16
SCALE = 1.0 / 8.0  # 1/sqrt(D)


def build_program():
    nc = bacc.Bacc(
        "TRN2",
        target_bir_lowering=False,
        debug=False,
        num_devices=NCORES,
    )
    xt_d = nc.dram_tensor("xt", [C, TOK], BF16, kind="ExternalInput").ap()
    wqkv_d = nc.dram_tensor("wqkv", [C, 3 * QKVC], BF16, kind="ExternalInput").ap()
    bqkv_d = nc.dram_tensor("bqkv", [3 * QKVC], F32, kind="ExternalInput").ap()
    wproj_d = nc.dram_tensor("wproj", [QKVC, C], F32R, kind="ExternalInput").ap()
    outp_d = nc.dram_tensor("outp", [TOK, C], F32, kind="ExternalOutput").ap()

    with tile.TileContext(nc) as tc:
        with ExitStack() as ctx, nc.allow_low_precision(reason="fp32r matmul inputs"):
            _body(ctx, tc, xt_d, wqkv_d, bqkv_d, wproj_d, outp_d)
    nc.compile()
    return nc


class _Kern:
    def __init__(self, ctx, tc, xt_d, wqkv_d, bqkv_d, wproj_d, outp_d):
        nc = tc.nc
        self.nc = nc
        self.tc = tc
        self.outp_d = outp_d

        self.const = ctx.enter_context(tc.tile_pool(name="const", bufs=1))
        self.persist = ctx.enter_context(tc.tile_pool(name="persist", bufs=1))
        self.xt_pool = ctx.enter_context(tc.tile_pool(name="xt", bufs=2))
        self.vtmp_pool = ctx.enter_context(tc.tile_pool(name="vtmp", bufs=2))
        self.pt_pool = ctx.enter_context(tc.tile_pool(name="pt", bufs=8))
        self.yt_pool = ctx.enter_context(tc.tile_pool(name="yt", bufs=2))
        self.out_pool = ctx.enter_context(tc.tile_pool(name="osb", bufs=3))
        self.small_pool = ctx.enter_context(tc.tile_pool(name="small", bufs=4))

        self.ps_s = ctx.enter_context(tc.tile_pool(name="ps_s", bufs=2, space="PSUM"))
        self.ps_y = ctx.enter_context(tc.tile_pool(name="ps_y", bufs=2, space="PSUM"))
        self.ps_mm = ctx.enter_context(tc.tile_pool(name="ps_mm", bufs=2, space="PSUM"))

        # --- constants ---
        c = self.const
        # weight loads ride the ACT HWDGE ring (nc.scalar) so they don't
        # serialize with the xt streaming loads on the SP ring; the first
        # K-chunk is split out so the PE can start quickly
        self.wqkv_s = c.tile([128, 8, 3 * QKVC], BF16, tag="wqkv", name="wqkv_s")
        wqkv_r = wqkv_d.rearrange("(kc p) m -> p kc m", p=128)
        for kc in range(8):
            nc.scalar.dma_start(self.wqkv_s[:, kc : kc + 1, :], wqkv_r[:, kc : kc + 1, :])
        self.bqkv_s = c.tile([128, 3], F32, tag="bqkv", name="bqkv_s")
        nc.scalar.dma_start(self.bqkv_s[:], bqkv_d.rearrange("(m p) -> p m", p=128))
        self.wproj_s = c.tile([128, C], F32R, tag="wproj", name="wproj_s")
        nc.scalar.dma_start(self.wproj_s[:], wproj_d[:])
        self.ident = c.tile([128, 128], F32, tag="ident", name="ident")
        make_identity(nc, self.ident[:])

        # persistent activations
        self.qT = self.persist.tile([128, TOK], F32R, tag="qT", name="qT")
        self.kT = self.persist.tile([128, TOK], F32R, tag="kT", name="kT")
        # token-major V (cols 0:D) + 64 replicated ones columns (cols D:2D):
        # the y^T matmul then yields the softmax denominator replicated on
        # psum partitions D..2D, row-aligned with y for the normalize mul
        self.vones = self.persist.tile(
            [128, B, HPC, NKC, 2 * D], BF16, tag="vones", name="vones"
        )
        nc.gpsimd.memset(self.vones[:, :, :, :, D : 2 * D], 1.0)
        self.xt_r = xt_d.rearrange("(kc p) t -> p kc t", p=128)

    def qkv_block(self, b, nb):
        """QKV + V-transpose for token block nb (TB tokens) of batch b."""
        nc = self.nc
        n = b * NBB + nb
        xt_t = self.xt_pool.tile([128, 8, TB], BF16, tag="xt", name=f"xt{n}")
        if n == 0:
            # cold start: split the first load so the PE can start sooner
            for kc in range(8):
                nc.sync.dma_start(
                    xt_t[:, kc, :], self.xt_r[:, kc, n * TB : (n + 1) * TB]
                )
        else:
            nc.sync.dma_start(xt_t[:], self.xt_r[:, :, n * TB : (n + 1) * TB])
        for m in range(3):  # q, k, v row chunks
            ps = self.ps_mm.tile([128, TB], F32, tag="mm", name=f"qkvp{n}_{m}")
            for kc in range(8):
                nc.tensor.matmul(
                    ps[:],
                    self.wqkv_s[:, kc, m * 128 : (m + 1) * 128],
                    xt_t[:, kc, :],
                    start=(kc == 0),
                    stop=(kc == 7),
                )
            if m < 2:
                dst = (self.qT if m == 0 else self.kT)[:, n * TB : (n + 1) * TB]
                nc.vector.tensor_scalar_add(dst, ps[:], self.bqkv_s[:, m : m + 1])
            else:
                vt = self.vtmp_pool.tile([128, TB], F32, tag="vt", name=f"vt{n}")
                nc.vector.tensor_scalar_add(vt[:], ps[:], self.bqkv_s[:, 2:3])
                j0 = (TB // 128) * nb
                for jj in range(TB // 128):
                    pst = self.ps_mm.tile([128, 128], F32, tag="mm", name=f"tr{n}_{jj}")
                    nc.tensor.transpose(
                        pst[:], vt[:, jj * 128 : (jj + 1) * 128], self.ident[:]
                    )
                    nc.vector.tensor_copy(
                        self.vones[:, b, :, j0 + jj, 0:D],
                        pst[:].rearrange("p (h d) -> p h d", h=HPC),
                    )

    def attn_block(self, b, qb):
        """Attention + proj for query block qb (QB queries) of batch b."""
        nc = self.nc
        q0 = b * T + qb * QB
        nj = (qb + 1) * (QB // 128)  # key chunks attended by this block
        psy = [
            self.ps_y.tile([2 * D, QB], F32, tag="psy", name=f"psy{b}_{qb}_{h}")
            for h in range(HPC)
        ]
        for j in range(nj):  # key chunks of 128
            k0 = b * T + j * 128
            # diagonal trimming: for a diagonal chunk at offset d, queries
            # f < 128*d attend to no key in this chunk, so restrict all work
            # to the query range [f0, QB)
            d = j - (nj - 4)
            # cap the restriction at 256 live queries: below that, fp32r
            # matmuls drop to 4 cyc/row and the "saved" columns cost more
            # than computing them (the mask zeroes them regardless)
            f0 = min(128 * d, QB - 256) if d > 0 else 0
            # one 2-bank psum tile holds both heads' S^T for this chunk;
            # the two K=64 matmuls use partitions 0-63 / 64-127 -> different
            # PE row groups, issued adjacently so they can run concurrently
            ps2 = self.ps_s.tile([128, HPC, QB], F32, tag="s2", name=f"s{b}_{qb}_{j}")
            for h in range(HPC):
                nc.tensor.matmul(
                    ps2[:, h, f0:QB],
                    self.kT[h * D : (h + 1) * D, k0 : k0 + 128],
                    self.qT[h * D : (h + 1) * D, q0 + f0 : q0 + QB],
                    start=True,
                    stop=True,
                )
            pt = self.pt_pool.tile([128, HPC, QB], BF16, tag="pt", name=f"pt{b}_{qb}_{j}")
            nc.scalar.activation(pt[:, :, f0:QB], ps2[:, :, f0:QB], AF.Exp, scale=SCALE)
            if d >= 0:
                # mask only the 128-column window straddling the diagonal,
                # per head so the first yT matmul isn't gated on both
                cols = min(QB, 128 * (d + 1))
                for h in range(HPC):
                    nc.gpsimd.affine_select(
                        out=pt[:, h, f0:cols],
                        in_=pt[:, h, f0:cols],
                        base=QB * qb - 128 * j + f0,
                        channel_multiplier=-1,
                        pattern=[[1, cols - f0]],
                        compare_op=mybir.AluOpType.is_ge,
                        fill=0.0,
                    )
            for h in range(HPC):
                nc.tensor.matmul(
                    psy[h][:, f0:QB],
                    self.vones[:, b, h, j, :],
                    pt[:, h, f0:QB],
                    start=(j == 0),
                    stop=(j == nj - 1),
                )
        # normalize into yt (d-major, both heads stacked)
        yt = self.yt_pool.tile([128, QB], F32R, tag="yt", name=f"yt{b}_{qb}")
        for h in range(HPC):
            rec = self.small_pool.tile([D, QB], F32, tag="rec", name=f"rec{b}_{qb}_{h}")
            nc.vector.reciprocal(rec[:], psy[h][D : 2 * D, :])
            nc.vector.tensor_mul(yt[h * D : (h + 1) * D, :], psy[h][0:D, :], rec[:])
        return yt

    def proj_block(self, b, qb, yt):
        """Projection + output DMA for query block qb of batch b."""
        nc = self.nc
        q0 = b * T + qb * QB
        for tt in range(QB // 128):
            osb = self.out_pool.tile([128, C], F32, tag="osb", name=f"o{b}_{qb}_{tt}")
            for ncol in range(C // 512):
                po = self.ps_mm.tile([128, 512], F32, tag="mm", name=f"po{b}_{qb}_{tt}_{ncol}")
                nc.tensor.matmul(
                    po[:],
                    yt[:, tt * 128 : (tt + 1) * 128],
                    self.wproj_s[:, ncol * 512 : (ncol + 1) * 512],
                    start=True,
                    stop=True,
                )
                nc.vector.tensor_copy(osb[:, ncol * 512 : (ncol + 1) * 512], po[:])
            r0 = q0 + tt * 128
            nc.sync.dma_start(self.outp_d[r0 : r0 + 128, :], osb[:])


def _body(ctx, tc, xt_d, wqkv_d, bqkv_d, wproj_d, outp_d):
    k = _Kern(ctx, tc, xt_d, wqkv_d, bqkv_d, wproj_d, outp_d)
    # Software pipeline one batch deep: attention(b) interleaves with the
    # independent qkv(b+1) blocks so the PE always has ready matmuls while
    # ACT paces the softmax. proj is deferred one attention block so the PE
    # has ready work while the softmax-normalize chain completes.
    # qkv(0) is the prologue; batches 1..B-1 form a queue drained 2 blocks
    # per attention slot for the first half, then 1, so every attention
    # stretch (including the last batch's) has PE-dense qkv filler.
    pending = None
    for nb in range(NBB):
        k.qkv_block(0, nb)
    queue = [(b, nb) for b in range(1, B) for nb in range(NBB)]
    qi = 0
    nslots = B * (T // QB)
    for s in range(nslots):
        b, qb = s // (T // QB), s % (T // QB)
        want = 2 if s < nslots // 2 else 1
        # never emit attn before its qkv blocks: need batch b block 2qb+1
        need = 0 if b == 0 else (b - 1) * NBB + 2 * qb + 2
        while qi < len(queue) and (qi < need or want > 0):
            k.qkv_block(*queue[qi])
            qi += 1
            want -= 1
        yt = k.attn_block(b, qb)
        if pending is not None:
            k.proj_block(*pending)
        pending = (b, qb, yt)
    while qi < len(queue):
        k.qkv_block(*queue[qi])
        qi += 1
    k.proj_block(*pending)


_CACHED_NC = None


def _get_nc():
    global _CACHED_NC
    if _CACHED_NC is None:
        _CACHED_NC = build_program()
    return _CACHED_NC


def make_in_maps(x, W_attn, b_attn, W_proj):
    x = np.ascontiguousarray(np.asarray(x, dtype=np.float32))
    W_attn = np.asarray(W_attn, dtype=np.float32)
    b_attn = np.asarray(b_attn, dtype=np.float32)
    W_proj = np.asarray(W_proj, dtype=np.float32)
    import ml_dtypes

    xt = np.ascontiguousarray(x.reshape(TOK, C).T.astype(ml_dtypes.bfloat16))
    in_maps = []
    for c in range(NCORES):
        s = c * QKVC
        wq = W_attn[:, s : s + QKVC]
        wk = W_attn[:, C + s : C + s + QKVC]
        wv = W_attn[:, 2 * C + s : 2 * C + s + QKVC]
        wqkv = np.ascontiguousarray(
            np.concatenate([wq, wk, wv], axis=1).astype(ml_dtypes.bfloat16)
        )
        bq = b_attn[s : s + QKVC]
        bk = b_attn[C + s : C + s + QKVC]
        bv = b_attn[2 * C + s : 2 * C + s + QKVC]
        bqkv = np.ascontiguousarray(np.concatenate([bq, bk, bv]))
        wproj = np.ascontiguousarray(W_proj[s : s + QKVC, :])
        in_maps.append({"xt": xt, "wqkv": wqkv, "bqkv": bqkv, "wproj": wproj})
    return in_maps


def run(x, W_attn, b_attn, W_proj, b_proj, trace=False, **kwargs):
    nc = _get_nc()
    in_maps = make_in_maps(x, W_attn, b_attn, W_proj)
    res = run_bass_kernel_spmd(
        nc, in_maps, core_ids=list(range(NCORES)), trace=trace, **kwargs
    )
    acc = res.results[0]["outp"].astype(np.float32, copy=True)
    for c in range(1, NCORES):
        acc += res.results[c]["outp"]
    acc += np.asarray(b_proj, dtype=np.float32)[None, :]
    out = acc.reshape(B, T, C)
    return out, res


def kernel(x, W_attn, b_attn, W_proj, b_proj):
    out, _ = run(x, W_attn, b_attn, W_proj, b_proj, trace=False)
    return out


# revision 33
# speedup vs baseline: 1.0179x; 1.0005x over previous
"""Causal self-attention Trainium2 kernel.

B=4, T=2048, C=1024, H=16 heads, D=64. 8 NeuronCores, tensor-parallel over
heads: core c owns heads {2c, 2c+1}. Host pre-transposes x to xT [C, B*T],
column-shards W_attn / row-shards W_proj, sums the 8 partial outputs.

Device kernel (per core, SPMD), software-pipelined one batch deep so the
PE-heavy qkv matmuls fill the gaps of the ACT-paced attention stream:
  qkv:  qkvT[384, T] = W_core.T @ xT  (bf16 matmuls, K=C in 8 chunks);
        rows: [qA qB | kA kB | vA vB], 64 each. Bias added on eviction (DVE).
        v rows are PE-transposed to token-major V with 64 appended ones
        columns, so the y^T matmul emits the softmax denominators
        replicated on psum partitions 64-127, row-aligned with y.
  attn: S^T layout: S^T[keys,queries] = k @ q^T via matmul(lhsT=kT_chunk,
        rhs=qT_block, fp32r); the two heads' K=64 matmuls are issued
        adjacently so the PE can run them concurrently in different row
        groups. exp on ACT over both heads at once ([128, 2, 512] psum);
        causal mask via per-head column-restricted gpsimd affine_select;
        diagonal chunks restrict all work to the live query range.
        y^T[d, queries] accumulated via matmul(lhsT=[V|1s], rhs=P^T, bf16);
        normalization is recip + row-aligned mul on DVE.
  proj: partial out[tokens, C] = y^T.T @ W_proj_rows (fp32r), DMA'd out;
        deferred one block so the PE has ready work during normalize.
"""

import sys

sys.path.insert(0, "/opt/trn_rl_repo")

from contextlib import ExitStack

import numpy as np

import concourse.bass as bass
import concourse.mybir as mybir
import concourse.tile as tile
from concourse import bacc
from concourse.bass_utils import run_bass_kernel_spmd
from concourse.masks import make_identity

F32 = mybir.dt.float32
F32R = mybir.dt.float32r
BF16 = mybir.dt.bfloat16
AF = mybir.ActivationFunctionType

B, T, C, H, D = 4, 2048, 1024, 16, 64
NCORES = 8
HPC = H // NCORES  # heads per core = 2
TOK = B * T  # 8192
QKVC = HPC * D  # per-core channels per q/k/v = 128
TB = 256  # token block for the qkv phase
NBB = T // TB  # qkv token blocks per batch = 8
QB = 512  # query block for attention
NKC = T // 128  # key chunks per batch = 16
SCALE = 1.0 / 8.0  # 1/sqrt(D)


def build_program():
    nc = bacc.Bacc(
        "TRN2",
        target_bir_lowering=False,
        debug=False,
        num_devices=NCORES,
    )
    xt_d = nc.dram_tensor("xt", [C, TOK], BF16, kind="ExternalInput").ap()
    wqkv_d = nc.dram_tensor("wqkv", [C, 3 * QKVC], BF16, kind="ExternalInput").ap()
    bqkv_d = nc.dram_tensor("bqkv", [3 * QKVC], F32, kind="ExternalInput").ap()
    wproj_d = nc.dram_tensor("wproj", [QKVC, C], F32R, kind="ExternalInput").ap()
    outp_d = nc.dram_tensor("outp", [TOK, C], F32, kind="ExternalOutput").ap()

    with tile.TileContext(nc) as tc:
        with ExitStack() as ctx, nc.allow_low_precision(reason="fp32r matmul inputs"):
            _body(ctx, tc, xt_d, wqkv_d, bqkv_d, wproj_d, outp_d)
    nc.compile()
    return nc


class _Kern:
    def __init__(self, ctx, tc, xt_d, wqkv_d, bqkv_d, wproj_d, outp_d):
        nc = tc.nc
        self.nc = nc
        self.tc = tc
        self.outp_d = outp_d

        self.const = ctx.enter_context(tc.tile_pool(name="const", bufs=1))
        self.persist = ctx.enter_context(tc.tile_pool(name="persist", bufs=1))
        self.xt_pool = ctx.enter_context(tc.tile_pool(name="xt", bufs=3))
        self.vtmp_pool = ctx.enter_context(tc.tile_pool(name="vtmp", bufs=3))
        self.pt_pool = ctx.enter_context(tc.tile_pool(name="pt", bufs=8))
        self.yt_pool = ctx.enter_context(tc.tile_pool(name="yt", bufs=3))
        self.out_pool = ctx.enter_context(tc.tile_pool(name="osb", bufs=4))
        self.small_pool = ctx.enter_context(tc.tile_pool(name="small", bufs=4))

        self.ps_s = ctx.enter_context(tc.tile_pool(name="ps_s", bufs=2, space="PSUM"))
        self.ps_y = ctx.enter_context(tc.tile_pool(name="ps_y", bufs=2, space="PSUM"))
        self.ps_mm = ctx.enter_context(tc.tile_pool(name="ps_mm", bufs=2, space="PSUM"))

        # --- constants ---
        c = self.const
        # weight loads ride the ACT HWDGE ring (nc.scalar) so they don't
        # serialize with the xt streaming loads on the SP ring; the first
        # K-chunk is split out so the PE can start quickly
        self.wqkv_s = c.tile([128, 8, 3 * QKVC], BF16, tag="wqkv", name="wqkv_s")
        wqkv_r = wqkv_d.rearrange("(kc p) m -> p kc m", p=128)
        for kc in range(8):
            nc.scalar.dma_start(self.wqkv_s[:, kc : kc + 1, :], wqkv_r[:, kc : kc + 1, :])
        self.bqkv_s = c.tile([128, 3], F32, tag="bqkv", name="bqkv_s")
        nc.scalar.dma_start(self.bqkv_s[:], bqkv_d.rearrange("(m p) -> p m", p=128))
        self.wproj_s = c.tile([128, C], F32R, tag="wproj", name="wproj_s")
        nc.scalar.dma_start(self.wproj_s[:], wproj_d[:])
        self.ident = c.tile([128, 128], F32, tag="ident", name="ident")
        make_identity(nc, self.ident[:])

        # persistent activations
        self.qT = self.persist.tile([128, TOK], F32R, tag="qT", name="qT")
        self.kT = self.persist.tile([128, TOK], F32R, tag="kT", name="kT")
        # token-major V (cols 0:D) + 64 replicated ones columns (cols D:2D):
        # the y^T matmul then yields the softmax denominator replicated on
        # psum partitions D..2D, row-aligned with y for the normalize mul
        self.vones = self.persist.tile(
            [128, B, HPC, NKC, 2 * D], BF16, tag="vones", name="vones"
        )
        nc.gpsimd.memset(self.vones[:, :, :, :, D : 2 * D], 1.0)
        self.xt_r = xt_d.rearrange("(kc p) t -> p kc t", p=128)

    def qkv_block(self, b, nb):
        """QKV + V-transpose for token block nb (TB tokens) of batch b."""
        nc = self.nc
        n = b * NBB + nb
        xt_t = self.xt_pool.tile([128, 8, TB], BF16, tag="xt", name=f"xt{n}")
        if n == 0:
            # cold start: split the first load so the PE can start sooner
            for kc in range(8):
                nc.sync.dma_start(
                    xt_t[:, kc, :], self.xt_r[:, kc, n * TB : (n + 1) * TB]
                )
        else:
            nc.sync.dma_start(xt_t[:], self.xt_r[:, :, n * TB : (n + 1) * TB])
        for m in range(3):  # q, k, v row chunks
            ps = self.ps_mm.tile([128, TB], F32, tag="mm", name=f"qkvp{n}_{m}")
            for kc in range(8):
                nc.tensor.matmul(
                    ps[:],
                    self.wqkv_s[:, kc, m * 128 : (m + 1) * 128],
                    xt_t[:, kc, :],
                    start=(kc == 0),
                    stop=(kc == 7),
                )
            if m < 2:
                dst = (self.qT if m == 0 else self.kT)[:, n * TB : (n + 1) * TB]
                nc.vector.tensor_scalar_add(dst, ps[:], self.bqkv_s[:, m : m + 1])
            else:
                vt = self.vtmp_pool.tile([128, TB], F32, tag="vt", name=f"vt{n}")
                nc.vector.tensor_scalar_add(vt[:], ps[:], self.bqkv_s[:, 2:3])
                j0 = (TB // 128) * nb
                for jj in range(TB // 128):
                    pst = self.ps_mm.tile([128, 128], F32, tag="mm", name=f"tr{n}_{jj}")
                    nc.tensor.transpose(
                        pst[:], vt[:, jj * 128 : (jj + 1) * 128], self.ident[:]
                    )
                    nc.vector.tensor_copy(
                        self.vones[:, b, :, j0 + jj, 0:D],
                        pst[:].rearrange("p (h d) -> p h d", h=HPC),
                    )

    def attn_block(self, b, qb):
        """Attention + proj for query block qb (QB queries) of batch b."""
        nc = self.nc
        q0 = b * T + qb * QB
        nj = (qb + 1) * (QB // 128)  # key chunks attended by this block
        psy = [
            self.ps_y.tile([2 * D, QB], F32, tag="psy", name=f"psy{b}_{qb}_{h}")
            for h in range(HPC)
        ]
        for j in range(nj):  # key chunks of 128
            k0 = b * T + j * 128
            # diagonal trimming: for a diagonal chunk at offset d, queries
            # f < 128*d attend to no key in this chunk, so restrict all work
            # to the query range [f0, QB)
            d = j - (nj - 4)
            # cap the restriction at 256 live queries: below that, fp32r
            # matmuls drop to 4 cyc/row and the "saved" columns cost more
            # than computing them (the mask zeroes them regardless)
            f0 = min(128 * d, QB - 256) if d > 0 else 0
            # one 2-bank psum tile holds both heads' S^T for this chunk;
            # the two K=64 matmuls use partitions 0-63 / 64-127 -> different
            # PE row groups, issued adjacently so they can run concurrently
            ps2 = self.ps_s.tile([128, HPC, QB], F32, tag="s2", name=f"s{b}_{qb}_{j}")
            for h in range(HPC):
                nc.tensor.matmul(
                    ps2[:, h, f0:QB],
                    self.kT[h * D : (h + 1) * D, k0 : k0 + 128],
                    self.qT[h * D : (h + 1) * D, q0 + f0 : q0 + QB],
                    start=True,
                    stop=True,
                )
            pt = self.pt_pool.tile([128, HPC, QB], BF16, tag="pt", name=f"pt{b}_{qb}_{j}")
            nc.scalar.activation(pt[:, :, f0:QB], ps2[:, :, f0:QB], AF.Exp, scale=SCALE)
            if d >= 0:
                # mask only the 128-column window straddling the diagonal,
                # per head so the first yT matmul isn't gated on both
                cols = min(QB, 128 * (d + 1))
                for h in range(HPC):
                    nc.gpsimd.affine_select(
                        out=pt[:, h, f0:cols],
                        in_=pt[:, h, f0:cols],
                        base=QB * qb - 128 * j + f0,
                        channel_multiplier=-1,
                        pattern=[[1, cols - f0]],
                        compare_op=mybir.AluOpType.is_ge,
                        fill=0.0,
                    )
            for h in range(HPC):
                nc.tensor.matmul(
                    psy[h][:, f0:QB],
                    self.vones[:, b, h, j, :],
                    pt[:, h, f0:QB],
                    start=(j == 0),
                    stop=(j == nj - 1),
                )
        # normalize into yt (d-major, both heads stacked)
        yt = self.yt_pool.tile([128, QB], F32R, tag="yt", name=f"yt{b}_{qb}")
        for h in range(HPC):
            rec = self.small_pool.tile([D, QB], F32, tag="rec", name=f"rec{b}_{qb}_{h}")
            nc.vector.reciprocal(rec[:], psy[h][D : 2 * D, :])
            nc.vector.tensor_mul(yt[h * D : (h + 1) * D, :], psy[h][0:D, :], rec[:])
        return yt

    def proj_block(self, b, qb, yt):
        """Projection + output DMA for query block qb of batch b."""
        nc = self.nc
        q0 = b * T + qb * QB
        for tt in range(QB // 128):
            osb = self.out_pool.tile([128, C], F32, tag="osb", name=f"o{b}_{qb}_{tt}")
            for ncol in range(C // 512):
                po = self.ps_mm.tile([128, 512], F32, tag="mm", name=f"po{b}_{qb}_{tt}_{ncol}")
                nc.tensor.matmul(
                    po[:],
                    yt[:, tt * 128 : (tt + 1) * 128],
                    self.wproj_s[:, ncol * 512 : (ncol + 1) * 512],
                    start=True,
                    stop=True,
                )
                nc.vector.tensor_copy(osb[:, ncol * 512 : (ncol + 1) * 512], po[:])
            r0 = q0 + tt * 128
            nc.sync.dma_start(self.outp_d[r0 : r0 + 128, :], osb[:])


def _body(ctx, tc, xt_d, wqkv_d, bqkv_d, wproj_d, outp_d):
    k = _Kern(ctx, tc, xt_d, wqkv_d, bqkv_d, wproj_d, outp_d)
    # Software pipeline one batch deep: attention(b) interleaves with the
    # independent qkv(b+1) blocks so the PE always has ready matmuls while
    # ACT paces the softmax. proj is deferred one attention block so the PE
    # has ready work while the softmax-normalize chain completes.
    # qkv(0) is the prologue; batches 1..B-1 form a queue drained 2 blocks
    # per attention slot for the first half, then 1, so every attention
    # stretch (including the last batch's) has PE-dense qkv filler.
    pending = None
    for nb in range(NBB):
        k.qkv_block(0, nb)
    queue = [(b, nb) for b in range(1, B) for nb in range(NBB)]
    qi = 0
    nslots = B * (T // QB)
    for s in range(nslots):
        b, qb = s // (T // QB), s % (T // QB)
        want = 2 if s < nslots // 2 else 1
        # never emit attn before its qkv blocks: need batch b block 2qb+1
        need = 0 if b == 0 else (b - 1) * NBB + 2 * qb + 2
        while qi < len(queue) and (qi < need or want > 0):
            k.qkv_block(*queue[qi])
            qi += 1
            want -= 1
        yt = k.attn_block(b, qb)
        if pending is not None:
            k.proj_block(*pending)
        pending = (b, qb, yt)
    while qi < len(queue):
        k.qkv_block(*queue[qi])
        qi += 1
    k.proj_block(*pending)


_CACHED_NC = None


def _get_nc():
    global _CACHED_NC
    if _CACHED_NC is None:
        _CACHED_NC = build_program()
    return _CACHED_NC


def make_in_maps(x, W_attn, b_attn, W_proj):
    x = np.ascontiguousarray(np.asarray(x, dtype=np.float32))
    W_attn = np.asarray(W_attn, dtype=np.float32)
    b_attn = np.asarray(b_attn, dtype=np.float32)
    W_proj = np.asarray(W_proj, dtype=np.float32)
    import ml_dtypes

    xt = np.ascontiguousarray(x.reshape(TOK, C).T.astype(ml_dtypes.bfloat16))
    in_maps = []
    for c in range(NCORES):
        s = c * QKVC
        wq = W_attn[:, s : s + QKVC]
        wk = W_attn[:, C + s : C + s + QKVC]
        wv = W_attn[:, 2 * C + s : 2 * C + s + QKVC]
        wqkv = np.ascontiguousarray(
            np.concatenate([wq, wk, wv], axis=1).astype(ml_dtypes.bfloat16)
        )
        bq = b_attn[s : s + QKVC]
        bk = b_attn[C + s : C + s + QKVC]
        bv = b_attn[2 * C + s : 2 * C + s + QKVC]
        bqkv = np.ascontiguousarray(np.concatenate([bq, bk, bv]))
        wproj = np.ascontiguousarray(W_proj[s : s + QKVC, :])
        in_maps.append({"xt": xt, "wqkv": wqkv, "bqkv": bqkv, "wproj": wproj})
    return in_maps


def run(x, W_attn, b_attn, W_proj, b_proj, trace=False, **kwargs):
    nc = _get_nc()
    in_maps = make_in_maps(x, W_attn, b_attn, W_proj)
    res = run_bass_kernel_spmd(
        nc, in_maps, core_ids=list(range(NCORES)), trace=trace, **kwargs
    )
    acc = res.results[0]["outp"].astype(np.float32, copy=True)
    for c in range(1, NCORES):
        acc += res.results[c]["outp"]
    acc += np.asarray(b_proj, dtype=np.float32)[None, :]
    out = acc.reshape(B, T, C)
    return out, res


def kernel(x, W_attn, b_attn, W_proj, b_proj):
    out, _ = run(x, W_attn, b_attn, W_proj, b_proj, trace=False)
    return out
